# revision 13
# baseline (speedup 1.0000x reference)
"""Bass/Trainium2 kernel for a heterogeneous-graph SAGEConv layer (DBGNNLayer).

Strategy (per the sharding hint: "shard edge lists and their gathered
messages across M devices"): the host gathers each edge's source-feature row
(scaled by the destination's 1/deg mean factor and the HeteroConv 0.5),
shards dst nodes across the 8 cores, and lays the gathered messages out in a
dense round-padded window format so the device kernel is pure streaming —
no dynamic DMA descriptors at all:

  * dst nodes of each type are sorted by degree and packed into windows of
    128; window w has a static round count K_w = max degree in the window
    (max over cores so the SPMD program is uniform).
  * the gathered messages for window w form a [128 feat, 128*K_w] bf16
    block: column d*K_w + t = (t-th neighbor message of dst d), zero-padded.
  * the device streams each block with one static DMA and computes the
    segment sum with a single strided vector-engine reduce:
        msgT[f, d] = sum_t pay[f, d*K_w + t]
  * out[d, :] = b + msgT.T @ Wl (+ msgT_tags.T @ Wl_tags) + x_dstT.T @ Wr
    via PSUM-accumulated bf16 matmuls (bias injected as a K=1 matmul with a
    ones row), then one PSUM->SBUF copy and a static DMA out.

All device data is bf16 (PSUM accumulation fp32); the host unpermutes the
window-sorted rows and upcasts to fp32.
"""

import sys

sys.path.insert(0, "/opt/trn_rl_repo")

import numpy as np

P = 128
NC_CORES = 8

_COMPILED_CACHE = {}


# ----------------------------------------------------------------- host utils

def _plan_windows(deg_a, deg_b=None):
    """Per-core degree-sorted window plan for one node type.

    deg_a/deg_b: [C, R] per-core degrees (b optional, e.g. tags for items).
    Returns (order [C, R] sorted dst index per core, Ka [NW], Kb [NW] or
    None) where Ka/Kb are max-over-cores per-window round counts.
    """
    C, R = deg_a.shape
    NW = -(-R // P)
    orders = np.empty((C, R), np.int64)
    Ka = np.zeros(NW, np.int64)
    Kb = np.zeros(NW, np.int64) if deg_b is not None else None
    for c in range(C):
        if deg_b is None:
            o = np.argsort(-deg_a[c], kind="stable")
        else:
            # coarse primary buckets so the secondary (tags) sort is
            # effective inside each bucket
            o = np.lexsort((-deg_b[c], -(deg_a[c] // 3)))
        orders[c] = o
        da = deg_a[c][o]
        for w in range(NW):
            seg = da[w * P:(w + 1) * P]
            Ka[w] = max(Ka[w], int(seg.max()) if len(seg) else 0)
        if deg_b is not None:
            db = deg_b[c][o]
            for w in range(NW):
                seg = db[w * P:(w + 1) * P]
                Kb[w] = max(Kb[w], int(seg.max()) if len(seg) else 0)
    return orders, Ka, (Kb if deg_b is not None else None)


def _build_payload(x_src, src, dst, n_dst, orders, K, recip, bf):
    """Build per-core transposed message payload [C, 128, SLOTS].

    Round-major column layout: off_w + t*128 + pos_in_window, i.e. window w
    is K_w round-blocks of [128 feat x 128 dst]; round t holds each dst's
    t-th edge message (zeros when deg < t).  The device segment-sum is then
    a halving tree of packed tensor_tensor adds over the round blocks.
    """
    C = NC_CORES
    R = n_dst // C
    NW = len(K)
    off = np.zeros(NW + 1, np.int64)
    np.cumsum(np.asarray(K) * P, out=off[1:])
    SLOTS = int(off[-1])

    # per-dst window/pos from orders
    win_of = np.empty(C * R, np.int64)
    pos_of = np.empty(C * R, np.int64)
    for c in range(C):
        o = orders[c]
        idx = np.arange(R)
        win_of[c * R + o] = idx // P
        pos_of[c * R + o] = idx % P

    core = dst // R
    # rank of each edge within its dst (arbitrary but stable order)
    order_e = np.argsort(dst, kind="stable")
    dst_s = dst[order_e]
    seg_start = np.zeros(n_dst + 1, np.int64)
    np.cumsum(np.bincount(dst_s, minlength=n_dst), out=seg_start[1:])
    rank_s = np.arange(len(dst)) - seg_start[dst_s]
    rank = np.empty(len(dst), np.int64)
    rank[order_e] = rank_s

    w = win_of[dst]
    col = off[w] + rank * P + pos_of[dst]
    gathered = (x_src[src] * recip[dst][:, None]).astype(bf)  # [E, 128]

    pay = np.zeros((C, SLOTS, P), bf)
    pay[core, col] = gathered
    payT = np.ascontiguousarray(pay.transpose(0, 2, 1))
    return payT, SLOTS


# ------------------------------------------------------------- device program

def _build_program(KU, KB, KT, NWU, NWI):
    import concourse.bacc as bacc
    import concourse.mybir as mybir
    from concourse import tile

    f32 = mybir.dt.float32
    bf16 = mybir.dt.bfloat16

    SLOTS_U = int(sum(KU)) * P
    SLOTS_B = int(sum(KB)) * P
    SLOTS_T = int(sum(KT)) * P

    nc = bacc.Bacc("TRN2", target_bir_lowering=False, debug=False,
                   enable_asserts=False, num_devices=NC_CORES)

    t_pu = nc.dram_tensor("pay_rev", [P, SLOTS_U], bf16, kind="ExternalInput")
    t_pb = nc.dram_tensor("pay_buys", [P, SLOTS_B], bf16,
                          kind="ExternalInput")
    t_pt = nc.dram_tensor("pay_tags", [P, SLOTS_T], bf16,
                          kind="ExternalInput")
    t_xdu = nc.dram_tensor("xdtu", [P, NWU * P], bf16, kind="ExternalInput")
    t_xdi = nc.dram_tensor("xdti", [P, NWI * P], bf16, kind="ExternalInput")
    # konst: Wl_rev | Wr_rev | Wl_buys | Wl_tags | Wr_item | ones | b_user
    # | b_item (the last three live in partition 0 only)
    t_const = nc.dram_tensor("konst", [P, 8 * P], bf16, kind="ExternalInput")
    t_ou = nc.dram_tensor("out_user", [NWU * P, P], bf16,
                          kind="ExternalOutput")
    t_oi = nc.dram_tensor("out_item", [NWI * P, P], bf16,
                          kind="ExternalOutput")

    with tile.TileContext(nc) as tc, \
         nc.allow_low_precision("bf16 segment-sum reduce; tolerance 2e-2"):
        with tc.tile_pool(name="const", bufs=1) as cpool, \
             tc.tile_pool(name="pay", bufs=6) as paypool, \
             tc.tile_pool(name="out", bufs=4) as opool, \
             tc.tile_pool(name="ps", bufs=4, space="PSUM") as ppool:
            konst = cpool.tile([P, 8 * P], bf16)
            nc.sync.dma_start(konst[:], t_const.ap())
            xdu = cpool.tile([P, NWU * P], bf16)
            nc.sync.dma_start(xdu[:], t_xdu.ap())
            xdi = cpool.tile([P, NWI * P], bf16)
            nc.sync.dma_start(xdi[:], t_xdi.ap())
            ones_row = konst[0:1, 5 * P:6 * P]

            def offsets(K, NW):
                o = np.zeros(NW + 1, np.int64)
                np.cumsum(np.asarray(K) * P, out=o[1:])
                return o

            off_u = offsets(KU, NWU)
            off_b = offsets(KB, NWI)
            off_t = offsets(KT, NWI)
            phase_u = dict(
                specs=[(t_pu, KU, off_u, konst[:, 0:P], "payu")],
                xd=xdu, b_row=konst[0:1, 6 * P:7 * P],
                wr_col=konst[:, P:2 * P], t_out=t_ou)
            phase_i = dict(
                specs=[(t_pb, KB, off_b, konst[:, 2 * P:3 * P], "payb"),
                       (t_pt, KT, off_t, konst[:, 3 * P:4 * P], "payt")],
                xd=xdi, b_row=konst[0:1, 7 * P:8 * P],
                wr_col=konst[:, 4 * P:5 * P], t_out=t_oi)

            def window(ph, w):
                msgs = []
                for si, (t_pay, K, offs, wl, tg) in enumerate(ph["specs"]):
                    kw = int(K[w])
                    if kw == 0:
                        continue
                    pay = paypool.tile([P, P * kw], bf16, tag=tg)
                    eng = nc.sync if (w + si) % 2 == 0 else nc.scalar
                    eng.dma_start(
                        pay[:],
                        t_pay.ap()[:, int(offs[w]):int(offs[w]) + P * kw])
                    # segment sum: halving tree of in-place packed adds over
                    # the kw round blocks, stopping at two blocks (folded
                    # into the PSUM-accumulated matmuls below)
                    k = kw
                    while k > 2:
                        if k % 2 == 1:
                            nc.vector.tensor_tensor(
                                out=pay[:, 0:P], in0=pay[:, 0:P],
                                in1=pay[:, (k - 1) * P:k * P],
                                op=mybir.AluOpType.add)
                            k -= 1
                        h = k // 2
                        nc.vector.tensor_tensor(
                            out=pay[:, 0:h * P], in0=pay[:, 0:h * P],
                            in1=pay[:, h * P:k * P],
                            op=mybir.AluOpType.add)
                        k = h
                    msgs.append((pay, k, wl))
                ps = ppool.tile([P, P], f32, space="PSUM", tag="out")
                nc.tensor.matmul(out=ps[:], lhsT=ones_row, rhs=ph["b_row"],
                                 start=True, stop=False)
                for pay, k, wl in msgs:
                    for t in range(k):
                        nc.tensor.matmul(out=ps[:],
                                         lhsT=pay[:, t * P:(t + 1) * P],
                                         rhs=wl, start=False, stop=False)
                nc.tensor.matmul(out=ps[:],
                                 lhsT=ph["xd"][:, w * P:(w + 1) * P],
                                 rhs=ph["wr_col"], start=False, stop=True)
                out_sb = opool.tile([P, P], bf16, tag="outsb")
                nc.scalar.copy(out=out_sb[:], in_=ps[:])
                nc.scalar.dma_start(
                    ph["t_out"].ap()[w * P:(w + 1) * P, :], out_sb[:])

            for w in range(max(NWU, NWI)):
                if w < NWI:
                    window(phase_i, w)
                if w < NWU:
                    window(phase_u, w)

    nc.compile()
    return nc


# ------------------------------------------------------------------- kernel()

def kernel(x_user, x_item, x_tag, ei_buys, ei_rev, ei_tags,
           Wl_buys, Wr_buys, b_buys,
           Wl_rev, Wr_rev, b_rev,
           Wl_tags, Wr_tags, b_tags):
    import ml_dtypes
    from concourse import bass_utils

    bf = ml_dtypes.bfloat16
    x_user = np.ascontiguousarray(np.asarray(x_user, np.float32))
    x_item = np.ascontiguousarray(np.asarray(x_item, np.float32))
    x_tag = np.ascontiguousarray(np.asarray(x_tag, np.float32))
    ei_buys = np.asarray(ei_buys, np.int64)
    ei_rev = np.asarray(ei_rev, np.int64)
    ei_tags = np.asarray(ei_tags, np.int64)

    n_user, n_item = x_user.shape[0], x_item.shape[0]
    C = NC_CORES
    ru, ri = n_user // C, n_item // C
    NWU, NWI = -(-ru // P), -(-ri // P)

    cnt_buys = np.bincount(ei_buys[1], minlength=n_item)
    cnt_rev = np.bincount(ei_rev[1], minlength=n_user)
    cnt_tags = np.bincount(ei_tags[1], minlength=n_item)
    r_buys = (0.5 / np.maximum(cnt_buys, 1)).astype(np.float32)
    r_rev = (1.0 / np.maximum(cnt_rev, 1)).astype(np.float32)
    r_tags = (0.5 / np.maximum(cnt_tags, 1)).astype(np.float32)

    ord_u, KU, _ = _plan_windows(cnt_rev.reshape(C, ru))
    ord_i, KB, KT = _plan_windows(cnt_buys.reshape(C, ri),
                                  cnt_tags.reshape(C, ri))

    pay_u, SU = _build_payload(x_item, ei_rev[0], ei_rev[1], n_user,
                               ord_u, KU, r_rev, bf)
    pay_b, SB = _build_payload(x_user, ei_buys[0], ei_buys[1], n_item,
                               ord_i, KB, r_buys, bf)
    pay_t, ST = _build_payload(x_tag, ei_tags[0], ei_tags[1], n_item,
                               ord_i, KT, r_tags, bf)

    # x_dst^T in window order, zero-padded to NW*P rows
    def xdt(x, orders, NW, R):
        out = np.zeros((C, P, NW * P), bf)
        for c in range(C):
            rows = x[c * R + orders[c]].astype(bf)      # [R, 128]
            out[c, :, :R] = rows.T
        return out

    xdtu = xdt(x_user, ord_u, NWU, ru)
    xdti = xdt(x_item, ord_i, NWI, ri)

    misc = np.zeros((P, 3 * P), np.float32)
    misc[0, 0:P] = 1.0
    misc[0, P:2 * P] = np.asarray(b_rev, np.float32)
    misc[0, 2 * P:3 * P] = 0.5 * (np.asarray(b_buys, np.float32)
                                  + np.asarray(b_tags, np.float32))
    konst = np.concatenate([
        np.asarray(Wl_rev, np.float32), np.asarray(Wr_rev, np.float32),
        np.asarray(Wl_buys, np.float32), np.asarray(Wl_tags, np.float32),
        0.5 * (np.asarray(Wr_buys, np.float32)
               + np.asarray(Wr_tags, np.float32)),
        misc,
    ], axis=1).astype(bf)

    key = (tuple(KU), tuple(KB), tuple(KT), NWU, NWI)
    if key not in _COMPILED_CACHE:
        _COMPILED_CACHE[key] = _build_program(*key)
    nc = _COMPILED_CACHE[key]

    in_maps = []
    for c in range(C):
        in_maps.append(dict(
            pay_rev=pay_u[c], pay_buys=pay_b[c], pay_tags=pay_t[c],
            xdtu=xdtu[c], xdti=xdti[c], konst=konst,
        ))

    res = bass_utils.run_bass_kernel_spmd(
        nc, in_maps, core_ids=list(range(C)))

    out_user = np.empty((n_user, P), np.float32)
    out_item = np.empty((n_item, P), np.float32)
    for c in range(C):
        ou = np.asarray(res.results[c]["out_user"], np.float32)
        oi = np.asarray(res.results[c]["out_item"], np.float32)
        out_user[c * ru + ord_u[c]] = ou[:ru]
        out_item[c * ri + ord_i[c]] = oi[:ri]
    return out_user, out_item


# revision 14
# speedup vs baseline: 1.0042x; 1.0042x over previous
"""Bass/Trainium2 kernel for a heterogeneous-graph SAGEConv layer (DBGNNLayer).

Strategy (per the sharding hint: "shard edge lists and their gathered
messages across M devices"): the host gathers each edge's source-feature row
(scaled by the destination's 1/deg mean factor and the HeteroConv 0.5),
shards dst nodes across the 8 cores, and lays the gathered messages out in a
dense round-padded window format so the device kernel is pure streaming —
no dynamic DMA descriptors at all:

  * dst nodes of each type are sorted by degree and packed into windows of
    128; window w has a static round count K_w = max degree in the window
    (max over cores so the SPMD program is uniform).
  * the gathered messages for window w form a [128 feat, 128*K_w] bf16
    block: column d*K_w + t = (t-th neighbor message of dst d), zero-padded.
  * the device streams each block with one static DMA and computes the
    segment sum with a single strided vector-engine reduce:
        msgT[f, d] = sum_t pay[f, d*K_w + t]
  * out[d, :] = b + msgT.T @ Wl (+ msgT_tags.T @ Wl_tags) + x_dstT.T @ Wr
    via PSUM-accumulated bf16 matmuls (bias injected as a K=1 matmul with a
    ones row), then one PSUM->SBUF copy and a static DMA out.

All device data is bf16 (PSUM accumulation fp32); the host unpermutes the
window-sorted rows and upcasts to fp32.
"""

import sys

sys.path.insert(0, "/opt/trn_rl_repo")

import numpy as np

P = 128
NC_CORES = 8

_COMPILED_CACHE = {}


# ----------------------------------------------------------------- host utils

def _plan_windows(deg_a, deg_b=None):
    """Per-core degree-sorted window plan for one node type.

    deg_a/deg_b: [C, R] per-core degrees (b optional, e.g. tags for items).
    Returns (order [C, R] sorted dst index per core, Ka [NW], Kb [NW] or
    None) where Ka/Kb are max-over-cores per-window round counts.
    """
    C, R = deg_a.shape
    NW = -(-R // P)
    orders = np.empty((C, R), np.int64)
    Ka = np.zeros(NW, np.int64)
    Kb = np.zeros(NW, np.int64) if deg_b is not None else None
    for c in range(C):
        if deg_b is None:
            o = np.argsort(-deg_a[c], kind="stable")
        else:
            # coarse primary buckets so the secondary (tags) sort is
            # effective inside each bucket
            o = np.lexsort((-deg_b[c], -(deg_a[c] // 3)))
        orders[c] = o
        da = deg_a[c][o]
        for w in range(NW):
            seg = da[w * P:(w + 1) * P]
            Ka[w] = max(Ka[w], int(seg.max()) if len(seg) else 0)
        if deg_b is not None:
            db = deg_b[c][o]
            for w in range(NW):
                seg = db[w * P:(w + 1) * P]
                Kb[w] = max(Kb[w], int(seg.max()) if len(seg) else 0)
    return orders, Ka, (Kb if deg_b is not None else None)


def _build_payload(x_src, src, dst, n_dst, orders, K, recip, bf):
    """Build per-core transposed message payload [C, 128, SLOTS].

    Round-major column layout: off_w + t*128 + pos_in_window, i.e. window w
    is K_w round-blocks of [128 feat x 128 dst]; round t holds each dst's
    t-th edge message (zeros when deg < t).  The device segment-sum is then
    a halving tree of packed tensor_tensor adds over the round blocks.
    """
    C = NC_CORES
    R = n_dst // C
    NW = len(K)
    off = np.zeros(NW + 1, np.int64)
    np.cumsum(np.asarray(K) * P, out=off[1:])
    SLOTS = int(off[-1])

    # per-dst window/pos from orders
    win_of = np.empty(C * R, np.int64)
    pos_of = np.empty(C * R, np.int64)
    for c in range(C):
        o = orders[c]
        idx = np.arange(R)
        win_of[c * R + o] = idx // P
        pos_of[c * R + o] = idx % P

    core = dst // R
    # rank of each edge within its dst (arbitrary but stable order)
    order_e = np.argsort(dst, kind="stable")
    dst_s = dst[order_e]
    seg_start = np.zeros(n_dst + 1, np.int64)
    np.cumsum(np.bincount(dst_s, minlength=n_dst), out=seg_start[1:])
    rank_s = np.arange(len(dst)) - seg_start[dst_s]
    rank = np.empty(len(dst), np.int64)
    rank[order_e] = rank_s

    w = win_of[dst]
    col = off[w] + rank * P + pos_of[dst]
    gathered = (x_src[src] * recip[dst][:, None]).astype(bf)  # [E, 128]

    pay = np.zeros((C, SLOTS, P), bf)
    pay[core, col] = gathered
    payT = np.ascontiguousarray(pay.transpose(0, 2, 1))
    return payT, SLOTS


# ------------------------------------------------------------- device program

def _build_program(KU, KB, KT, NWU, NWI):
    import concourse.bacc as bacc
    import concourse.mybir as mybir
    from concourse import tile

    f32 = mybir.dt.float32
    bf16 = mybir.dt.bfloat16

    SLOTS_U = int(sum(KU)) * P
    SLOTS_B = int(sum(KB)) * P
    SLOTS_T = int(sum(KT)) * P

    nc = bacc.Bacc("TRN2", target_bir_lowering=False, debug=False,
                   enable_asserts=False, num_devices=NC_CORES)

    t_pu = nc.dram_tensor("pay_rev", [P, SLOTS_U], bf16, kind="ExternalInput")
    t_pb = nc.dram_tensor("pay_buys", [P, SLOTS_B], bf16,
                          kind="ExternalInput")
    t_pt = nc.dram_tensor("pay_tags", [P, SLOTS_T], bf16,
                          kind="ExternalInput")
    t_xdu = nc.dram_tensor("xdtu", [P, NWU * P], bf16, kind="ExternalInput")
    t_xdi = nc.dram_tensor("xdti", [P, NWI * P], bf16, kind="ExternalInput")
    # konst: Wl_rev | Wr_rev | Wl_buys | Wl_tags | Wr_item | ones | b_user
    # | b_item (the last three live in partition 0 only)
    t_const = nc.dram_tensor("konst", [P, 8 * P], bf16, kind="ExternalInput")
    t_ou = nc.dram_tensor("out_user", [NWU * P, P], bf16,
                          kind="ExternalOutput")
    t_oi = nc.dram_tensor("out_item", [NWI * P, P], bf16,
                          kind="ExternalOutput")

    with tile.TileContext(nc) as tc, \
         nc.allow_low_precision("bf16 segment-sum reduce; tolerance 2e-2"):
        with tc.tile_pool(name="const", bufs=1) as cpool, \
             tc.tile_pool(name="pay", bufs=6) as paypool, \
             tc.tile_pool(name="out", bufs=4) as opool, \
             tc.tile_pool(name="ps", bufs=4, space="PSUM") as ppool:
            konst = cpool.tile([P, 8 * P], bf16)
            nc.sync.dma_start(konst[:], t_const.ap())
            xdu = cpool.tile([P, NWU * P], bf16)
            nc.sync.dma_start(xdu[:], t_xdu.ap())
            xdi = cpool.tile([P, NWI * P], bf16)
            nc.sync.dma_start(xdi[:], t_xdi.ap())
            ones_row = konst[0:1, 5 * P:6 * P]

            def offsets(K, NW):
                o = np.zeros(NW + 1, np.int64)
                np.cumsum(np.asarray(K) * P, out=o[1:])
                return o

            off_u = offsets(KU, NWU)
            off_b = offsets(KB, NWI)
            off_t = offsets(KT, NWI)
            phase_u = dict(
                specs=[(t_pu, KU, off_u, konst[:, 0:P], "payu")],
                xd=xdu, b_row=konst[0:1, 6 * P:7 * P],
                wr_col=konst[:, P:2 * P], t_out=t_ou)
            phase_i = dict(
                specs=[(t_pb, KB, off_b, konst[:, 2 * P:3 * P], "payb"),
                       (t_pt, KT, off_t, konst[:, 3 * P:4 * P], "payt")],
                xd=xdi, b_row=konst[0:1, 7 * P:8 * P],
                wr_col=konst[:, 4 * P:5 * P], t_out=t_oi)

            def window(ph, w):
                msgs = []
                for si, (t_pay, K, offs, wl, tg) in enumerate(ph["specs"]):
                    kw = int(K[w])
                    if kw == 0:
                        continue
                    pay = paypool.tile([P, P * kw], bf16, tag=tg)
                    nc.sync.dma_start(
                        pay[:],
                        t_pay.ap()[:, int(offs[w]):int(offs[w]) + P * kw])
                    # segment sum: halving tree of in-place packed adds over
                    # the kw round blocks, stopping at two blocks (folded
                    # into the PSUM-accumulated matmuls below)
                    k = kw
                    while k > 2:
                        if k % 2 == 1:
                            nc.vector.tensor_tensor(
                                out=pay[:, 0:P], in0=pay[:, 0:P],
                                in1=pay[:, (k - 1) * P:k * P],
                                op=mybir.AluOpType.add)
                            k -= 1
                        h = k // 2
                        nc.vector.tensor_tensor(
                            out=pay[:, 0:h * P], in0=pay[:, 0:h * P],
                            in1=pay[:, h * P:k * P],
                            op=mybir.AluOpType.add)
                        k = h
                    msgs.append((pay, k, wl))
                ps = ppool.tile([P, P], f32, space="PSUM", tag="out")
                nc.tensor.matmul(out=ps[:], lhsT=ones_row, rhs=ph["b_row"],
                                 start=True, stop=False)
                for pay, k, wl in msgs:
                    for t in range(k):
                        nc.tensor.matmul(out=ps[:],
                                         lhsT=pay[:, t * P:(t + 1) * P],
                                         rhs=wl, start=False, stop=False)
                nc.tensor.matmul(out=ps[:],
                                 lhsT=ph["xd"][:, w * P:(w + 1) * P],
                                 rhs=ph["wr_col"], start=False, stop=True)
                out_sb = opool.tile([P, P], bf16, tag="outsb")
                nc.scalar.copy(out=out_sb[:], in_=ps[:])
                nc.scalar.dma_start(
                    ph["t_out"].ap()[w * P:(w + 1) * P, :], out_sb[:])

            for w in range(max(NWU, NWI)):
                if w < NWI:
                    window(phase_i, w)
                if w < NWU:
                    window(phase_u, w)

    nc.compile()
    return nc


# ------------------------------------------------------------------- kernel()

def kernel(x_user, x_item, x_tag, ei_buys, ei_rev, ei_tags,
           Wl_buys, Wr_buys, b_buys,
           Wl_rev, Wr_rev, b_rev,
           Wl_tags, Wr_tags, b_tags):
    import ml_dtypes
    from concourse import bass_utils

    bf = ml_dtypes.bfloat16
    x_user = np.ascontiguousarray(np.asarray(x_user, np.float32))
    x_item = np.ascontiguousarray(np.asarray(x_item, np.float32))
    x_tag = np.ascontiguousarray(np.asarray(x_tag, np.float32))
    ei_buys = np.asarray(ei_buys, np.int64)
    ei_rev = np.asarray(ei_rev, np.int64)
    ei_tags = np.asarray(ei_tags, np.int64)

    n_user, n_item = x_user.shape[0], x_item.shape[0]
    C = NC_CORES
    ru, ri = n_user // C, n_item // C
    NWU, NWI = -(-ru // P), -(-ri // P)

    cnt_buys = np.bincount(ei_buys[1], minlength=n_item)
    cnt_rev = np.bincount(ei_rev[1], minlength=n_user)
    cnt_tags = np.bincount(ei_tags[1], minlength=n_item)
    r_buys = (0.5 / np.maximum(cnt_buys, 1)).astype(np.float32)
    r_rev = (1.0 / np.maximum(cnt_rev, 1)).astype(np.float32)
    r_tags = (0.5 / np.maximum(cnt_tags, 1)).astype(np.float32)

    ord_u, KU, _ = _plan_windows(cnt_rev.reshape(C, ru))
    ord_i, KB, KT = _plan_windows(cnt_buys.reshape(C, ri),
                                  cnt_tags.reshape(C, ri))

    pay_u, SU = _build_payload(x_item, ei_rev[0], ei_rev[1], n_user,
                               ord_u, KU, r_rev, bf)
    pay_b, SB = _build_payload(x_user, ei_buys[0], ei_buys[1], n_item,
                               ord_i, KB, r_buys, bf)
    pay_t, ST = _build_payload(x_tag, ei_tags[0], ei_tags[1], n_item,
                               ord_i, KT, r_tags, bf)

    # x_dst^T in window order, zero-padded to NW*P rows
    def xdt(x, orders, NW, R):
        out = np.zeros((C, P, NW * P), bf)
        for c in range(C):
            rows = x[c * R + orders[c]].astype(bf)      # [R, 128]
            out[c, :, :R] = rows.T
        return out

    xdtu = xdt(x_user, ord_u, NWU, ru)
    xdti = xdt(x_item, ord_i, NWI, ri)

    misc = np.zeros((P, 3 * P), np.float32)
    misc[0, 0:P] = 1.0
    misc[0, P:2 * P] = np.asarray(b_rev, np.float32)
    misc[0, 2 * P:3 * P] = 0.5 * (np.asarray(b_buys, np.float32)
                                  + np.asarray(b_tags, np.float32))
    konst = np.concatenate([
        np.asarray(Wl_rev, np.float32), np.asarray(Wr_rev, np.float32),
        np.asarray(Wl_buys, np.float32), np.asarray(Wl_tags, np.float32),
        0.5 * (np.asarray(Wr_buys, np.float32)
               + np.asarray(Wr_tags, np.float32)),
        misc,
    ], axis=1).astype(bf)

    key = (tuple(KU), tuple(KB), tuple(KT), NWU, NWI)
    if key not in _COMPILED_CACHE:
        _COMPILED_CACHE[key] = _build_program(*key)
    nc = _COMPILED_CACHE[key]

    in_maps = []
    for c in range(C):
        in_maps.append(dict(
            pay_rev=pay_u[c], pay_buys=pay_b[c], pay_tags=pay_t[c],
            xdtu=xdtu[c], xdti=xdti[c], konst=konst,
        ))

    res = bass_utils.run_bass_kernel_spmd(
        nc, in_maps, core_ids=list(range(C)))

    out_user = np.empty((n_user, P), np.float32)
    out_item = np.empty((n_item, P), np.float32)
    for c in range(C):
        ou = np.asarray(res.results[c]["out_user"], np.float32)
        oi = np.asarray(res.results[c]["out_item"], np.float32)
        out_user[c * ru + ord_u[c]] = ou[:ru]
        out_item[c * ri + ord_i[c]] = oi[:ri]
    return out_user, out_item


# revision 15
# speedup vs baseline: 1.0218x; 1.0176x over previous
"""Bass/Trainium2 kernel for a heterogeneous-graph SAGEConv layer (DBGNNLayer).

Strategy (per the sharding hint: "shard edge lists and their gathered
messages across M devices"): the host gathers each edge's source-feature row
(scaled by the destination's 1/deg mean factor and the HeteroConv 0.5),
shards dst nodes across the 8 cores, and lays the gathered messages out in a
dense round-padded window format so the device kernel is pure streaming —
no dynamic DMA descriptors at all:

  * dst nodes of each type are sorted by degree and packed into windows of
    128; window w has a static round count K_w = max degree in the window
    (max over cores so the SPMD program is uniform).
  * the gathered messages for window w form a [128 feat, 128*K_w] bf16
    block: column d*K_w + t = (t-th neighbor message of dst d), zero-padded.
  * the device streams each block with one static DMA and computes the
    segment sum with a single strided vector-engine reduce:
        msgT[f, d] = sum_t pay[f, d*K_w + t]
  * out[d, :] = b + msgT.T @ Wl (+ msgT_tags.T @ Wl_tags) + x_dstT.T @ Wr
    via PSUM-accumulated bf16 matmuls (bias injected as a K=1 matmul with a
    ones row), then one PSUM->SBUF copy and a static DMA out.

All device data is bf16 (PSUM accumulation fp32); the host unpermutes the
window-sorted rows and upcasts to fp32.
"""

import sys

sys.path.insert(0, "/opt/trn_rl_repo")

import numpy as np

P = 128
NC_CORES = 8

_COMPILED_CACHE = {}


# ----------------------------------------------------------------- host utils

def _plan_windows(deg_a, deg_b=None):
    """Per-core degree-sorted window plan for one node type.

    deg_a/deg_b: [C, R] per-core degrees (b optional, e.g. tags for items).
    Returns (order [C, R] sorted dst index per core, Ka [NW], Kb [NW] or
    None) where Ka/Kb are max-over-cores per-window round counts.
    """
    C, R = deg_a.shape
    NW = -(-R // P)
    orders = np.empty((C, R), np.int64)
    Ka = np.zeros(NW, np.int64)
    Kb = np.zeros(NW, np.int64) if deg_b is not None else None
    for c in range(C):
        if deg_b is None:
            o = np.argsort(-deg_a[c], kind="stable")
        else:
            # coarse primary buckets so the secondary (tags) sort is
            # effective inside each bucket
            o = np.lexsort((-deg_b[c], -(deg_a[c] // 3)))
        orders[c] = o
        da = deg_a[c][o]
        for w in range(NW):
            seg = da[w * P:(w + 1) * P]
            Ka[w] = max(Ka[w], int(seg.max()) if len(seg) else 0)
        if deg_b is not None:
            db = deg_b[c][o]
            for w in range(NW):
                seg = db[w * P:(w + 1) * P]
                Kb[w] = max(Kb[w], int(seg.max()) if len(seg) else 0)
    # round K up to even: keeps the halving tree free of leading odd-fix
    # steps (shorter serial chains per window)
    Ka += Ka % 2
    if Kb is not None:
        Kb += Kb % 2
    return orders, Ka, (Kb if deg_b is not None else None)


def _build_payload(x_src, src, dst, n_dst, orders, K, recip, bf):
    """Build per-core transposed message payload [C, 128, SLOTS].

    Round-major column layout: off_w + t*128 + pos_in_window, i.e. window w
    is K_w round-blocks of [128 feat x 128 dst]; round t holds each dst's
    t-th edge message (zeros when deg < t).  The device segment-sum is then
    a halving tree of packed tensor_tensor adds over the round blocks.
    """
    C = NC_CORES
    R = n_dst // C
    NW = len(K)
    off = np.zeros(NW + 1, np.int64)
    np.cumsum(np.asarray(K) * P, out=off[1:])
    SLOTS = int(off[-1])

    # per-dst window/pos from orders
    win_of = np.empty(C * R, np.int64)
    pos_of = np.empty(C * R, np.int64)
    for c in range(C):
        o = orders[c]
        idx = np.arange(R)
        win_of[c * R + o] = idx // P
        pos_of[c * R + o] = idx % P

    core = dst // R
    # rank of each edge within its dst (arbitrary but stable order)
    order_e = np.argsort(dst, kind="stable")
    dst_s = dst[order_e]
    seg_start = np.zeros(n_dst + 1, np.int64)
    np.cumsum(np.bincount(dst_s, minlength=n_dst), out=seg_start[1:])
    rank_s = np.arange(len(dst)) - seg_start[dst_s]
    rank = np.empty(len(dst), np.int64)
    rank[order_e] = rank_s

    w = win_of[dst]
    col = off[w] + rank * P + pos_of[dst]
    gathered = (x_src[src] * recip[dst][:, None]).astype(bf)  # [E, 128]

    pay = np.zeros((C, SLOTS, P), bf)
    pay[core, col] = gathered
    payT = np.ascontiguousarray(pay.transpose(0, 2, 1))
    return payT, SLOTS


# ------------------------------------------------------------- device program

def _build_program(KU, KB, KT, NWU, NWI):
    import concourse.bacc as bacc
    import concourse.mybir as mybir
    from concourse import tile

    f32 = mybir.dt.float32
    bf16 = mybir.dt.bfloat16

    SLOTS_U = int(sum(KU)) * P
    SLOTS_B = int(sum(KB)) * P
    SLOTS_T = int(sum(KT)) * P

    nc = bacc.Bacc("TRN2", target_bir_lowering=False, debug=False,
                   enable_asserts=False, num_devices=NC_CORES)

    t_pu = nc.dram_tensor("pay_rev", [P, SLOTS_U], bf16, kind="ExternalInput")
    t_pb = nc.dram_tensor("pay_buys", [P, SLOTS_B], bf16,
                          kind="ExternalInput")
    t_pt = nc.dram_tensor("pay_tags", [P, SLOTS_T], bf16,
                          kind="ExternalInput")
    t_xdu = nc.dram_tensor("xdtu", [P, NWU * P], bf16, kind="ExternalInput")
    t_xdi = nc.dram_tensor("xdti", [P, NWI * P], bf16, kind="ExternalInput")
    # konst: Wl_rev | Wr_rev | Wl_buys | Wl_tags | Wr_item | ones | b_user
    # | b_item (the last three live in partition 0 only)
    t_const = nc.dram_tensor("konst", [P, 8 * P], bf16, kind="ExternalInput")
    t_ou = nc.dram_tensor("out_user", [NWU * P, P], bf16,
                          kind="ExternalOutput")
    t_oi = nc.dram_tensor("out_item", [NWI * P, P], bf16,
                          kind="ExternalOutput")

    with tile.TileContext(nc) as tc, \
         nc.allow_low_precision("bf16 segment-sum reduce; tolerance 2e-2"):
        with tc.tile_pool(name="const", bufs=1) as cpool, \
             tc.tile_pool(name="pay", bufs=6) as paypool, \
             tc.tile_pool(name="out", bufs=4) as opool, \
             tc.tile_pool(name="ps", bufs=4, space="PSUM") as ppool:
            konst = cpool.tile([P, 8 * P], bf16)
            nc.sync.dma_start(konst[:], t_const.ap())
            xdu = cpool.tile([P, NWU * P], bf16)
            nc.sync.dma_start(xdu[:], t_xdu.ap())
            xdi = cpool.tile([P, NWI * P], bf16)
            nc.sync.dma_start(xdi[:], t_xdi.ap())
            ones_row = konst[0:1, 5 * P:6 * P]

            def offsets(K, NW):
                o = np.zeros(NW + 1, np.int64)
                np.cumsum(np.asarray(K) * P, out=o[1:])
                return o

            off_u = offsets(KU, NWU)
            off_b = offsets(KB, NWI)
            off_t = offsets(KT, NWI)
            phase_u = dict(
                specs=[(t_pu, KU, off_u, konst[:, 0:P], "payu")],
                xd=xdu, b_row=konst[0:1, 6 * P:7 * P],
                wr_col=konst[:, P:2 * P], t_out=t_ou)
            phase_i = dict(
                specs=[(t_pb, KB, off_b, konst[:, 2 * P:3 * P], "payb"),
                       (t_pt, KT, off_t, konst[:, 3 * P:4 * P], "payt")],
                xd=xdi, b_row=konst[0:1, 7 * P:8 * P],
                wr_col=konst[:, 4 * P:5 * P], t_out=t_oi)

            def window(ph, w):
                msgs = []
                for si, (t_pay, K, offs, wl, tg) in enumerate(ph["specs"]):
                    kw = int(K[w])
                    if kw == 0:
                        continue
                    pay = paypool.tile([P, P * kw], bf16, tag=tg)
                    nc.sync.dma_start(
                        pay[:],
                        t_pay.ap()[:, int(offs[w]):int(offs[w]) + P * kw])
                    # segment sum: halving tree of in-place packed adds over
                    # the kw round blocks, stopping at two blocks (folded
                    # into the PSUM-accumulated matmuls below)
                    k = kw
                    while k > 2:
                        if k % 2 == 1:
                            nc.vector.tensor_tensor(
                                out=pay[:, 0:P], in0=pay[:, 0:P],
                                in1=pay[:, (k - 1) * P:k * P],
                                op=mybir.AluOpType.add)
                            k -= 1
                        h = k // 2
                        nc.vector.tensor_tensor(
                            out=pay[:, 0:h * P], in0=pay[:, 0:h * P],
                            in1=pay[:, h * P:k * P],
                            op=mybir.AluOpType.add)
                        k = h
                    msgs.append((pay, k, wl))
                ps = ppool.tile([P, P], f32, space="PSUM", tag="out")
                nc.tensor.matmul(out=ps[:], lhsT=ones_row, rhs=ph["b_row"],
                                 start=True, stop=False)
                for pay, k, wl in msgs:
                    for t in range(k):
                        nc.tensor.matmul(out=ps[:],
                                         lhsT=pay[:, t * P:(t + 1) * P],
                                         rhs=wl, start=False, stop=False)
                nc.tensor.matmul(out=ps[:],
                                 lhsT=ph["xd"][:, w * P:(w + 1) * P],
                                 rhs=ph["wr_col"], start=False, stop=True)
                out_sb = opool.tile([P, P], bf16, tag="outsb")
                nc.scalar.copy(out=out_sb[:], in_=ps[:])
                nc.scalar.dma_start(
                    ph["t_out"].ap()[w * P:(w + 1) * P, :], out_sb[:])

            for w in range(max(NWU, NWI)):
                if w < NWI:
                    window(phase_i, w)
                if w < NWU:
                    window(phase_u, w)

    nc.compile()
    return nc


# ------------------------------------------------------------------- kernel()

def kernel(x_user, x_item, x_tag, ei_buys, ei_rev, ei_tags,
           Wl_buys, Wr_buys, b_buys,
           Wl_rev, Wr_rev, b_rev,
           Wl_tags, Wr_tags, b_tags):
    import ml_dtypes
    from concourse import bass_utils

    bf = ml_dtypes.bfloat16
    x_user = np.ascontiguousarray(np.asarray(x_user, np.float32))
    x_item = np.ascontiguousarray(np.asarray(x_item, np.float32))
    x_tag = np.ascontiguousarray(np.asarray(x_tag, np.float32))
    ei_buys = np.asarray(ei_buys, np.int64)
    ei_rev = np.asarray(ei_rev, np.int64)
    ei_tags = np.asarray(ei_tags, np.int64)

    n_user, n_item = x_user.shape[0], x_item.shape[0]
    C = NC_CORES
    ru, ri = n_user // C, n_item // C
    NWU, NWI = -(-ru // P), -(-ri // P)

    cnt_buys = np.bincount(ei_buys[1], minlength=n_item)
    cnt_rev = np.bincount(ei_rev[1], minlength=n_user)
    cnt_tags = np.bincount(ei_tags[1], minlength=n_item)
    r_buys = (0.5 / np.maximum(cnt_buys, 1)).astype(np.float32)
    r_rev = (1.0 / np.maximum(cnt_rev, 1)).astype(np.float32)
    r_tags = (0.5 / np.maximum(cnt_tags, 1)).astype(np.float32)

    ord_u, KU, _ = _plan_windows(cnt_rev.reshape(C, ru))
    ord_i, KB, KT = _plan_windows(cnt_buys.reshape(C, ri),
                                  cnt_tags.reshape(C, ri))

    pay_u, SU = _build_payload(x_item, ei_rev[0], ei_rev[1], n_user,
                               ord_u, KU, r_rev, bf)
    pay_b, SB = _build_payload(x_user, ei_buys[0], ei_buys[1], n_item,
                               ord_i, KB, r_buys, bf)
    pay_t, ST = _build_payload(x_tag, ei_tags[0], ei_tags[1], n_item,
                               ord_i, KT, r_tags, bf)

    # x_dst^T in window order, zero-padded to NW*P rows
    def xdt(x, orders, NW, R):
        out = np.zeros((C, P, NW * P), bf)
        for c in range(C):
            rows = x[c * R + orders[c]].astype(bf)      # [R, 128]
            out[c, :, :R] = rows.T
        return out

    xdtu = xdt(x_user, ord_u, NWU, ru)
    xdti = xdt(x_item, ord_i, NWI, ri)

    misc = np.zeros((P, 3 * P), np.float32)
    misc[0, 0:P] = 1.0
    misc[0, P:2 * P] = np.asarray(b_rev, np.float32)
    misc[0, 2 * P:3 * P] = 0.5 * (np.asarray(b_buys, np.float32)
                                  + np.asarray(b_tags, np.float32))
    konst = np.concatenate([
        np.asarray(Wl_rev, np.float32), np.asarray(Wr_rev, np.float32),
        np.asarray(Wl_buys, np.float32), np.asarray(Wl_tags, np.float32),
        0.5 * (np.asarray(Wr_buys, np.float32)
               + np.asarray(Wr_tags, np.float32)),
        misc,
    ], axis=1).astype(bf)

    key = (tuple(KU), tuple(KB), tuple(KT), NWU, NWI)
    if key not in _COMPILED_CACHE:
        _COMPILED_CACHE[key] = _build_program(*key)
    nc = _COMPILED_CACHE[key]

    in_maps = []
    for c in range(C):
        in_maps.append(dict(
            pay_rev=pay_u[c], pay_buys=pay_b[c], pay_tags=pay_t[c],
            xdtu=xdtu[c], xdti=xdti[c], konst=konst,
        ))

    res = bass_utils.run_bass_kernel_spmd(
        nc, in_maps, core_ids=list(range(C)))

    out_user = np.empty((n_user, P), np.float32)
    out_item = np.empty((n_item, P), np.float32)
    for c in range(C):
        ou = np.asarray(res.results[c]["out_user"], np.float32)
        oi = np.asarray(res.results[c]["out_item"], np.float32)
        out_user[c * ru + ord_u[c]] = ou[:ru]
        out_item[c * ri + ord_i[c]] = oi[:ri]
    return out_user, out_item


# revision 17
# speedup vs baseline: 1.0316x; 1.0095x over previous
"""Bass/Trainium2 kernel for a heterogeneous-graph SAGEConv layer (DBGNNLayer).

Strategy (per the sharding hint: "shard edge lists and their gathered
messages across M devices"): the host gathers each edge's source-feature row
(scaled by the destination's 1/deg mean factor and the HeteroConv 0.5),
shards dst nodes across the 8 cores, and lays the gathered messages out in a
dense round-padded window format so the device kernel is pure streaming —
no dynamic DMA descriptors at all:

  * dst nodes of each type are sorted by degree and packed into windows of
    128; window w has a static round count K_w = max degree in the window
    (max over cores so the SPMD program is uniform).
  * the gathered messages for window w form a [128 feat, 128*K_w] bf16
    block of K_w round-major [128 feat x 128 dst] sub-blocks; round t holds
    each dst's t-th neighbor message (zero-padded past the degree).
  * the device streams each block with one static DMA and computes the
    segment sum as a halving tree of in-place packed tensor_tensor adds
    over the round blocks (DVE 2x mode), stopping at two blocks — the last
    add rides along as an extra PSUM-accumulating matmul.
  * out[d, :] = b + msgT.T @ Wl (+ msgT_tags.T @ Wl_tags) + x_dstT.T @ Wr
    via PSUM-accumulated bf16 matmuls (bias injected as a K=1 matmul with a
    ones row), then one PSUM->SBUF copy and a static DMA out; user and item
    windows are interleaved in one loop to keep all engines loaded.

All device data is bf16 (PSUM accumulation fp32); the host unpermutes the
window-sorted rows and upcasts to fp32.
"""

import sys

sys.path.insert(0, "/opt/trn_rl_repo")

import numpy as np

P = 128
NC_CORES = 8

_COMPILED_CACHE = {}


# ----------------------------------------------------------------- host utils

def _plan_windows(deg_a, deg_b=None):
    """Per-core degree-sorted window plan for one node type.

    deg_a/deg_b: [C, R] per-core degrees (b optional, e.g. tags for items).
    Returns (order [C, R] sorted dst index per core, Ka [NW], Kb [NW] or
    None) where Ka/Kb are max-over-cores per-window round counts.
    """
    C, R = deg_a.shape
    NW = -(-R // P)
    orders = np.empty((C, R), np.int64)
    Ka = np.zeros(NW, np.int64)
    Kb = np.zeros(NW, np.int64) if deg_b is not None else None
    for c in range(C):
        if deg_b is None:
            o = np.argsort(-deg_a[c], kind="stable")
        else:
            # coarse primary buckets so the secondary (tags) sort is
            # effective inside each bucket
            o = np.lexsort((-deg_b[c], -(deg_a[c] // 3)))
        orders[c] = o
        da = deg_a[c][o]
        for w in range(NW):
            seg = da[w * P:(w + 1) * P]
            Ka[w] = max(Ka[w], int(seg.max()) if len(seg) else 0)
        if deg_b is not None:
            db = deg_b[c][o]
            for w in range(NW):
                seg = db[w * P:(w + 1) * P]
                Kb[w] = max(Kb[w], int(seg.max()) if len(seg) else 0)
    # round K up to even: keeps the halving tree free of leading odd-fix
    # steps (shorter serial chains per window)
    Ka += Ka % 2
    if Kb is not None:
        Kb += Kb % 2
    return orders, Ka, (Kb if deg_b is not None else None)


def _build_payload(x_src, src, dst, n_dst, orders, K, recip, bf):
    """Build per-core transposed message payload [C, 128, SLOTS].

    Round-major column layout: off_w + t*128 + pos_in_window, i.e. window w
    is K_w round-blocks of [128 feat x 128 dst]; round t holds each dst's
    t-th edge message (zeros when deg < t).  The device segment-sum is then
    a halving tree of packed tensor_tensor adds over the round blocks.
    """
    C = NC_CORES
    R = n_dst // C
    NW = len(K)
    off = np.zeros(NW + 1, np.int64)
    np.cumsum(np.asarray(K) * P, out=off[1:])
    SLOTS = int(off[-1])

    # per-dst window/pos from orders
    win_of = np.empty(C * R, np.int64)
    pos_of = np.empty(C * R, np.int64)
    for c in range(C):
        o = orders[c]
        idx = np.arange(R)
        win_of[c * R + o] = idx // P
        pos_of[c * R + o] = idx % P

    core = dst // R
    # rank of each edge within its dst (arbitrary but stable order)
    order_e = np.argsort(dst, kind="stable")
    dst_s = dst[order_e]
    seg_start = np.zeros(n_dst + 1, np.int64)
    np.cumsum(np.bincount(dst_s, minlength=n_dst), out=seg_start[1:])
    rank_s = np.arange(len(dst)) - seg_start[dst_s]
    rank = np.empty(len(dst), np.int64)
    rank[order_e] = rank_s

    w = win_of[dst]
    col = off[w] + rank * P + pos_of[dst]
    gathered = (x_src[src] * recip[dst][:, None]).astype(bf)  # [E, 128]

    pay = np.zeros((C, SLOTS, P), bf)
    pay[core, col] = gathered
    payT = np.ascontiguousarray(pay.transpose(0, 2, 1))
    return payT, SLOTS


# ------------------------------------------------------------- device program

def _build_program(KU, KB, KT, NWU, NWI):
    import concourse.bacc as bacc
    import concourse.mybir as mybir
    from concourse import tile

    f32 = mybir.dt.float32
    bf16 = mybir.dt.bfloat16

    SLOTS_U = int(sum(KU)) * P
    SLOTS_B = int(sum(KB)) * P
    SLOTS_T = int(sum(KT)) * P

    nc = bacc.Bacc("TRN2", target_bir_lowering=False, debug=False,
                   enable_asserts=False, num_devices=NC_CORES)

    t_pu = nc.dram_tensor("pay_rev", [P, SLOTS_U], bf16, kind="ExternalInput")
    t_pb = nc.dram_tensor("pay_buys", [P, SLOTS_B], bf16,
                          kind="ExternalInput")
    t_pt = nc.dram_tensor("pay_tags", [P, SLOTS_T], bf16,
                          kind="ExternalInput")
    t_xdu = nc.dram_tensor("xdtu", [P, NWU * P], bf16, kind="ExternalInput")
    t_xdi = nc.dram_tensor("xdti", [P, NWI * P], bf16, kind="ExternalInput")
    # konst: Wl_rev | Wr_rev | Wl_buys | Wl_tags | Wr_item | ones | b_user
    # | b_item (the last three live in partition 0 only)
    t_const = nc.dram_tensor("konst", [P, 8 * P], bf16, kind="ExternalInput")
    t_ou = nc.dram_tensor("out_user", [NWU * P, P], bf16,
                          kind="ExternalOutput")
    t_oi = nc.dram_tensor("out_item", [NWI * P, P], bf16,
                          kind="ExternalOutput")

    with tile.TileContext(nc) as tc, \
         nc.allow_low_precision("bf16 segment-sum reduce; tolerance 2e-2"):
        with tc.tile_pool(name="const", bufs=1) as cpool, \
             tc.tile_pool(name="pay", bufs=5) as paypool, \
             tc.tile_pool(name="out", bufs=4) as opool, \
             tc.tile_pool(name="ps", bufs=4, space="PSUM") as ppool:
            konst = cpool.tile([P, 8 * P], bf16)
            nc.sync.dma_start(konst[:], t_const.ap())
            xdu = cpool.tile([P, NWU * P], bf16)
            nc.sync.dma_start(xdu[:], t_xdu.ap())
            xdi = cpool.tile([P, NWI * P], bf16)
            nc.sync.dma_start(xdi[:], t_xdi.ap())
            ones_row = konst[0:1, 5 * P:6 * P]

            def offsets(K, NW):
                o = np.zeros(NW + 1, np.int64)
                np.cumsum(np.asarray(K) * P, out=o[1:])
                return o

            off_u = offsets(KU, NWU)
            off_b = offsets(KB, NWI)
            off_t = offsets(KT, NWI)
            phase_u = dict(
                specs=[(t_pu, KU, off_u, konst[:, 0:P], "payu")],
                xd=xdu, b_row=konst[0:1, 6 * P:7 * P],
                wr_col=konst[:, P:2 * P], t_out=t_ou)
            phase_i = dict(
                specs=[(t_pb, KB, off_b, konst[:, 2 * P:3 * P], "payb"),
                       (t_pt, KT, off_t, konst[:, 3 * P:4 * P], "payt")],
                xd=xdi, b_row=konst[0:1, 7 * P:8 * P],
                wr_col=konst[:, 4 * P:5 * P], t_out=t_oi)

            def window(ph, w):
                msgs = []
                for si, (t_pay, K, offs, wl, tg) in enumerate(ph["specs"]):
                    kw = int(K[w])
                    if kw == 0:
                        continue
                    pay = paypool.tile([P, P * kw], bf16, tag=tg)
                    nc.sync.dma_start(
                        pay[:],
                        t_pay.ap()[:, int(offs[w]):int(offs[w]) + P * kw])
                    # segment sum: halving tree of in-place packed adds over
                    # the kw round blocks, stopping at two blocks (folded
                    # into the PSUM-accumulated matmuls below)
                    k = kw
                    while k > 2:
                        if k % 2 == 1:
                            nc.vector.tensor_tensor(
                                out=pay[:, 0:P], in0=pay[:, 0:P],
                                in1=pay[:, (k - 1) * P:k * P],
                                op=mybir.AluOpType.add)
                            k -= 1
                        h = k // 2
                        nc.vector.tensor_tensor(
                            out=pay[:, 0:h * P], in0=pay[:, 0:h * P],
                            in1=pay[:, h * P:k * P],
                            op=mybir.AluOpType.add)
                        k = h
                    msgs.append((pay, k, wl))
                ps = ppool.tile([P, P], f32, space="PSUM", tag="out")
                nc.tensor.matmul(out=ps[:], lhsT=ones_row, rhs=ph["b_row"],
                                 start=True, stop=False)
                for pay, k, wl in msgs:
                    for t in range(k):
                        nc.tensor.matmul(out=ps[:],
                                         lhsT=pay[:, t * P:(t + 1) * P],
                                         rhs=wl, start=False, stop=False)
                nc.tensor.matmul(out=ps[:],
                                 lhsT=ph["xd"][:, w * P:(w + 1) * P],
                                 rhs=ph["wr_col"], start=False, stop=True)
                out_sb = opool.tile([P, P], bf16, tag="outsb")
                nc.scalar.copy(out=out_sb[:], in_=ps[:])
                nc.scalar.dma_start(
                    ph["t_out"].ap()[w * P:(w + 1) * P, :], out_sb[:])

            for w in range(max(NWU, NWI)):
                if w < NWI:
                    window(phase_i, w)
                if w < NWU:
                    window(phase_u, w)

    nc.compile()
    return nc


# ------------------------------------------------------------------- kernel()

def kernel(x_user, x_item, x_tag, ei_buys, ei_rev, ei_tags,
           Wl_buys, Wr_buys, b_buys,
           Wl_rev, Wr_rev, b_rev,
           Wl_tags, Wr_tags, b_tags):
    import ml_dtypes
    from concourse import bass_utils

    bf = ml_dtypes.bfloat16
    x_user = np.ascontiguousarray(np.asarray(x_user, np.float32))
    x_item = np.ascontiguousarray(np.asarray(x_item, np.float32))
    x_tag = np.ascontiguousarray(np.asarray(x_tag, np.float32))
    ei_buys = np.asarray(ei_buys, np.int64)
    ei_rev = np.asarray(ei_rev, np.int64)
    ei_tags = np.asarray(ei_tags, np.int64)

    n_user, n_item = x_user.shape[0], x_item.shape[0]
    C = NC_CORES
    ru, ri = n_user // C, n_item // C
    NWU, NWI = -(-ru // P), -(-ri // P)

    cnt_buys = np.bincount(ei_buys[1], minlength=n_item)
    cnt_rev = np.bincount(ei_rev[1], minlength=n_user)
    cnt_tags = np.bincount(ei_tags[1], minlength=n_item)
    r_buys = (0.5 / np.maximum(cnt_buys, 1)).astype(np.float32)
    r_rev = (1.0 / np.maximum(cnt_rev, 1)).astype(np.float32)
    r_tags = (0.5 / np.maximum(cnt_tags, 1)).astype(np.float32)

    ord_u, KU, _ = _plan_windows(cnt_rev.reshape(C, ru))
    ord_i, KB, KT = _plan_windows(cnt_buys.reshape(C, ri),
                                  cnt_tags.reshape(C, ri))

    pay_u, SU = _build_payload(x_item, ei_rev[0], ei_rev[1], n_user,
                               ord_u, KU, r_rev, bf)
    pay_b, SB = _build_payload(x_user, ei_buys[0], ei_buys[1], n_item,
                               ord_i, KB, r_buys, bf)
    pay_t, ST = _build_payload(x_tag, ei_tags[0], ei_tags[1], n_item,
                               ord_i, KT, r_tags, bf)

    # x_dst^T in window order, zero-padded to NW*P rows
    def xdt(x, orders, NW, R):
        out = np.zeros((C, P, NW * P), bf)
        for c in range(C):
            rows = x[c * R + orders[c]].astype(bf)      # [R, 128]
            out[c, :, :R] = rows.T
        return out

    xdtu = xdt(x_user, ord_u, NWU, ru)
    xdti = xdt(x_item, ord_i, NWI, ri)

    misc = np.zeros((P, 3 * P), np.float32)
    misc[0, 0:P] = 1.0
    misc[0, P:2 * P] = np.asarray(b_rev, np.float32)
    misc[0, 2 * P:3 * P] = 0.5 * (np.asarray(b_buys, np.float32)
                                  + np.asarray(b_tags, np.float32))
    konst = np.concatenate([
        np.asarray(Wl_rev, np.float32), np.asarray(Wr_rev, np.float32),
        np.asarray(Wl_buys, np.float32), np.asarray(Wl_tags, np.float32),
        0.5 * (np.asarray(Wr_buys, np.float32)
               + np.asarray(Wr_tags, np.float32)),
        misc,
    ], axis=1).astype(bf)

    key = (tuple(KU), tuple(KB), tuple(KT), NWU, NWI)
    if key not in _COMPILED_CACHE:
        _COMPILED_CACHE[key] = _build_program(*key)
    nc = _COMPILED_CACHE[key]

    in_maps = []
    for c in range(C):
        in_maps.append(dict(
            pay_rev=pay_u[c], pay_buys=pay_b[c], pay_tags=pay_t[c],
            xdtu=xdtu[c], xdti=xdti[c], konst=konst,
        ))

    res = bass_utils.run_bass_kernel_spmd(
        nc, in_maps, core_ids=list(range(C)))

    out_user = np.empty((n_user, P), np.float32)
    out_item = np.empty((n_item, P), np.float32)
    for c in range(C):
        ou = np.asarray(res.results[c]["out_user"], np.float32)
        oi = np.asarray(res.results[c]["out_item"], np.float32)
        out_user[c * ru + ord_u[c]] = ou[:ru]
        out_item[c * ri + ord_i[c]] = oi[:ri]
    return out_user, out_item


# revision 18
# speedup vs baseline: 1.0565x; 1.0242x over previous
"""Bass/Trainium2 kernel for a heterogeneous-graph SAGEConv layer (DBGNNLayer).

Strategy (per the sharding hint: "shard edge lists and their gathered
messages across M devices"): the host gathers each edge's source-feature row
(scaled by the destination's 1/deg mean factor and the HeteroConv 0.5),
shards dst nodes across the 8 cores, and lays the gathered messages out in a
dense round-padded window format so the device kernel is pure streaming —
no dynamic DMA descriptors at all:

  * dst nodes of each type are sorted by degree and packed into windows of
    128; window w has a static round count K_w = max degree in the window
    (max over cores so the SPMD program is uniform).
  * the gathered messages for window w form a [128 feat, 128*K_w] bf16
    block of K_w round-major [128 feat x 128 dst] sub-blocks; round t holds
    each dst's t-th neighbor message (zero-padded past the degree).
  * the device streams each block with one static DMA and computes the
    segment sum as a halving tree of in-place packed tensor_tensor adds
    over the round blocks (DVE 2x mode), stopping at two blocks — the last
    add rides along as an extra PSUM-accumulating matmul.
  * out[d, :] = b + msgT.T @ Wl (+ msgT_tags.T @ Wl_tags) + x_dstT.T @ Wr
    via PSUM-accumulated bf16 matmuls (bias injected as a K=1 matmul with a
    ones row), then one PSUM->SBUF copy and a static DMA out; user and item
    windows are interleaved in one loop to keep all engines loaded.

All device data is bf16 (PSUM accumulation fp32); the host unpermutes the
window-sorted rows and upcasts to fp32.
"""

import sys

sys.path.insert(0, "/opt/trn_rl_repo")

import numpy as np

P = 128
NC_CORES = 8

_COMPILED_CACHE = {}


# ----------------------------------------------------------------- host utils

def _plan_windows(deg_a, deg_b=None):
    """Per-core degree-sorted window plan for one node type.

    deg_a/deg_b: [C, R] per-core degrees (b optional, e.g. tags for items).
    Returns (order [C, R] sorted dst index per core, Ka [NW], Kb [NW] or
    None) where Ka/Kb are max-over-cores per-window round counts.
    """
    C, R = deg_a.shape
    NW = -(-R // P)
    orders = np.empty((C, R), np.int64)
    Ka = np.zeros(NW, np.int64)
    Kb = np.zeros(NW, np.int64) if deg_b is not None else None
    for c in range(C):
        if deg_b is None:
            o = np.argsort(-deg_a[c], kind="stable")
        else:
            # coarse primary buckets so the secondary (tags) sort is
            # effective inside each bucket
            o = np.lexsort((-deg_b[c], -(deg_a[c] // 3)))
        orders[c] = o
        da = deg_a[c][o]
        for w in range(NW):
            seg = da[w * P:(w + 1) * P]
            Ka[w] = max(Ka[w], int(seg.max()) if len(seg) else 0)
        if deg_b is not None:
            db = deg_b[c][o]
            for w in range(NW):
                seg = db[w * P:(w + 1) * P]
                Kb[w] = max(Kb[w], int(seg.max()) if len(seg) else 0)
    # round K up to even: keeps the halving tree free of leading odd-fix
    # steps (shorter serial chains per window)
    Ka += Ka % 2
    if Kb is not None:
        Kb += Kb % 2
    return orders, Ka, (Kb if deg_b is not None else None)


def _build_payload(x_src, src, dst, n_dst, orders, K, recip, bf):
    """Build per-core transposed message payload [C, 128, SLOTS].

    Round-major column layout: off_w + t*128 + pos_in_window, i.e. window w
    is K_w round-blocks of [128 feat x 128 dst]; round t holds each dst's
    t-th edge message (zeros when deg < t).  The device segment-sum is then
    a halving tree of packed tensor_tensor adds over the round blocks.
    """
    C = NC_CORES
    R = n_dst // C
    NW = len(K)
    off = np.zeros(NW + 1, np.int64)
    np.cumsum(np.asarray(K) * P, out=off[1:])
    SLOTS = int(off[-1])

    # per-dst window/pos from orders
    win_of = np.empty(C * R, np.int64)
    pos_of = np.empty(C * R, np.int64)
    for c in range(C):
        o = orders[c]
        idx = np.arange(R)
        win_of[c * R + o] = idx // P
        pos_of[c * R + o] = idx % P

    core = dst // R
    # rank of each edge within its dst (arbitrary but stable order)
    order_e = np.argsort(dst, kind="stable")
    dst_s = dst[order_e]
    seg_start = np.zeros(n_dst + 1, np.int64)
    np.cumsum(np.bincount(dst_s, minlength=n_dst), out=seg_start[1:])
    rank_s = np.arange(len(dst)) - seg_start[dst_s]
    rank = np.empty(len(dst), np.int64)
    rank[order_e] = rank_s

    w = win_of[dst]
    col = off[w] + rank * P + pos_of[dst]
    gathered = (x_src[src] * recip[dst][:, None]).astype(bf)  # [E, 128]

    pay = np.zeros((C, SLOTS, P), bf)
    pay[core, col] = gathered
    payT = np.ascontiguousarray(pay.transpose(0, 2, 1))
    return payT, SLOTS


# ------------------------------------------------------------- device program

def _build_program(KU, KB, KT, NWU, NWI):
    import concourse.bacc as bacc
    import concourse.mybir as mybir
    from concourse import tile

    f32 = mybir.dt.float32
    bf16 = mybir.dt.bfloat16

    SLOTS_U = int(sum(KU)) * P
    SLOTS_B = int(sum(KB)) * P
    SLOTS_T = int(sum(KT)) * P

    nc = bacc.Bacc("TRN2", target_bir_lowering=False, debug=False,
                   enable_asserts=False, num_devices=NC_CORES)

    t_pu = nc.dram_tensor("pay_rev", [P, SLOTS_U], bf16, kind="ExternalInput")
    t_pb = nc.dram_tensor("pay_buys", [P, SLOTS_B], bf16,
                          kind="ExternalInput")
    t_pt = nc.dram_tensor("pay_tags", [P, SLOTS_T], bf16,
                          kind="ExternalInput")
    t_xdu = nc.dram_tensor("xdtu", [P, NWU * P], bf16, kind="ExternalInput")
    t_xdi = nc.dram_tensor("xdti", [P, NWI * P], bf16, kind="ExternalInput")
    # konst: Wl_rev | Wr_rev | Wl_buys | Wl_tags | Wr_item | ones | b_user
    # | b_item (the last three live in partition 0 only)
    t_const = nc.dram_tensor("konst", [P, 8 * P], bf16, kind="ExternalInput")
    t_ou = nc.dram_tensor("out_user", [NWU * P, P], bf16,
                          kind="ExternalOutput")
    t_oi = nc.dram_tensor("out_item", [NWI * P, P], bf16,
                          kind="ExternalOutput")

    with tile.TileContext(nc) as tc, \
         nc.allow_low_precision("bf16 segment-sum reduce; tolerance 2e-2"):
        with tc.tile_pool(name="const", bufs=1) as cpool, \
             tc.tile_pool(name="pay", bufs=5) as paypool, \
             tc.tile_pool(name="out", bufs=4) as opool, \
             tc.tile_pool(name="ps", bufs=4, space="PSUM") as ppool:
            konst = cpool.tile([P, 8 * P], bf16)
            nc.scalar.dma_start(konst[:], t_const.ap())
            xdu = cpool.tile([P, NWU * P], bf16)
            nc.scalar.dma_start(xdu[:], t_xdu.ap())
            xdi = cpool.tile([P, NWI * P], bf16)
            nc.scalar.dma_start(xdi[:], t_xdi.ap())
            ones_row = konst[0:1, 5 * P:6 * P]

            def offsets(K, NW):
                o = np.zeros(NW + 1, np.int64)
                np.cumsum(np.asarray(K) * P, out=o[1:])
                return o

            off_u = offsets(KU, NWU)
            off_b = offsets(KB, NWI)
            off_t = offsets(KT, NWI)
            phase_u = dict(
                specs=[(t_pu, KU, off_u, konst[:, 0:P], "payu")],
                xd=xdu, b_row=konst[0:1, 6 * P:7 * P],
                wr_col=konst[:, P:2 * P], t_out=t_ou)
            phase_i = dict(
                specs=[(t_pb, KB, off_b, konst[:, 2 * P:3 * P], "payb"),
                       (t_pt, KT, off_t, konst[:, 3 * P:4 * P], "payt")],
                xd=xdi, b_row=konst[0:1, 7 * P:8 * P],
                wr_col=konst[:, 4 * P:5 * P], t_out=t_oi)

            def window(ph, w):
                msgs = []
                for si, (t_pay, K, offs, wl, tg) in enumerate(ph["specs"]):
                    kw = int(K[w])
                    if kw == 0:
                        continue
                    pay = paypool.tile([P, P * kw], bf16, tag=tg)
                    nc.sync.dma_start(
                        pay[:],
                        t_pay.ap()[:, int(offs[w]):int(offs[w]) + P * kw])
                    # segment sum: halving tree of in-place packed adds over
                    # the kw round blocks, stopping at four blocks (folded
                    # into the PSUM-accumulated matmuls below)
                    k = kw
                    while k > 4:
                        if k % 2 == 1:
                            nc.vector.tensor_tensor(
                                out=pay[:, 0:P], in0=pay[:, 0:P],
                                in1=pay[:, (k - 1) * P:k * P],
                                op=mybir.AluOpType.add)
                            k -= 1
                        else:
                            h = k // 2
                            nc.vector.tensor_tensor(
                                out=pay[:, 0:h * P], in0=pay[:, 0:h * P],
                                in1=pay[:, h * P:k * P],
                                op=mybir.AluOpType.add)
                            k = h
                    msgs.append((pay, k, wl))
                ps = ppool.tile([P, P], f32, space="PSUM", tag="out")
                nc.tensor.matmul(out=ps[:], lhsT=ones_row, rhs=ph["b_row"],
                                 start=True, stop=False)
                for pay, k, wl in msgs:
                    for t in range(k):
                        nc.tensor.matmul(out=ps[:],
                                         lhsT=pay[:, t * P:(t + 1) * P],
                                         rhs=wl, start=False, stop=False)
                nc.tensor.matmul(out=ps[:],
                                 lhsT=ph["xd"][:, w * P:(w + 1) * P],
                                 rhs=ph["wr_col"], start=False, stop=True)
                out_sb = opool.tile([P, P], bf16, tag="outsb")
                nc.scalar.copy(out=out_sb[:], in_=ps[:])
                nc.scalar.dma_start(
                    ph["t_out"].ap()[w * P:(w + 1) * P, :], out_sb[:])

            for w in range(max(NWU, NWI)):
                if w < NWI:
                    window(phase_i, w)
                if w < NWU:
                    window(phase_u, w)

    nc.compile()
    return nc


# ------------------------------------------------------------------- kernel()

def kernel(x_user, x_item, x_tag, ei_buys, ei_rev, ei_tags,
           Wl_buys, Wr_buys, b_buys,
           Wl_rev, Wr_rev, b_rev,
           Wl_tags, Wr_tags, b_tags):
    import ml_dtypes
    from concourse import bass_utils

    bf = ml_dtypes.bfloat16
    x_user = np.ascontiguousarray(np.asarray(x_user, np.float32))
    x_item = np.ascontiguousarray(np.asarray(x_item, np.float32))
    x_tag = np.ascontiguousarray(np.asarray(x_tag, np.float32))
    ei_buys = np.asarray(ei_buys, np.int64)
    ei_rev = np.asarray(ei_rev, np.int64)
    ei_tags = np.asarray(ei_tags, np.int64)

    n_user, n_item = x_user.shape[0], x_item.shape[0]
    C = NC_CORES
    ru, ri = n_user // C, n_item // C
    NWU, NWI = -(-ru // P), -(-ri // P)

    cnt_buys = np.bincount(ei_buys[1], minlength=n_item)
    cnt_rev = np.bincount(ei_rev[1], minlength=n_user)
    cnt_tags = np.bincount(ei_tags[1], minlength=n_item)
    r_buys = (0.5 / np.maximum(cnt_buys, 1)).astype(np.float32)
    r_rev = (1.0 / np.maximum(cnt_rev, 1)).astype(np.float32)
    r_tags = (0.5 / np.maximum(cnt_tags, 1)).astype(np.float32)

    ord_u, KU, _ = _plan_windows(cnt_rev.reshape(C, ru))
    ord_i, KB, KT = _plan_windows(cnt_buys.reshape(C, ri),
                                  cnt_tags.reshape(C, ri))

    pay_u, SU = _build_payload(x_item, ei_rev[0], ei_rev[1], n_user,
                               ord_u, KU, r_rev, bf)
    pay_b, SB = _build_payload(x_user, ei_buys[0], ei_buys[1], n_item,
                               ord_i, KB, r_buys, bf)
    pay_t, ST = _build_payload(x_tag, ei_tags[0], ei_tags[1], n_item,
                               ord_i, KT, r_tags, bf)

    # x_dst^T in window order, zero-padded to NW*P rows
    def xdt(x, orders, NW, R):
        out = np.zeros((C, P, NW * P), bf)
        for c in range(C):
            rows = x[c * R + orders[c]].astype(bf)      # [R, 128]
            out[c, :, :R] = rows.T
        return out

    xdtu = xdt(x_user, ord_u, NWU, ru)
    xdti = xdt(x_item, ord_i, NWI, ri)

    misc = np.zeros((P, 3 * P), np.float32)
    misc[0, 0:P] = 1.0
    misc[0, P:2 * P] = np.asarray(b_rev, np.float32)
    misc[0, 2 * P:3 * P] = 0.5 * (np.asarray(b_buys, np.float32)
                                  + np.asarray(b_tags, np.float32))
    konst = np.concatenate([
        np.asarray(Wl_rev, np.float32), np.asarray(Wr_rev, np.float32),
        np.asarray(Wl_buys, np.float32), np.asarray(Wl_tags, np.float32),
        0.5 * (np.asarray(Wr_buys, np.float32)
               + np.asarray(Wr_tags, np.float32)),
        misc,
    ], axis=1).astype(bf)

    key = (tuple(KU), tuple(KB), tuple(KT), NWU, NWI)
    if key not in _COMPILED_CACHE:
        _COMPILED_CACHE[key] = _build_program(*key)
    nc = _COMPILED_CACHE[key]

    in_maps = []
    for c in range(C):
        in_maps.append(dict(
            pay_rev=pay_u[c], pay_buys=pay_b[c], pay_tags=pay_t[c],
            xdtu=xdtu[c], xdti=xdti[c], konst=konst,
        ))

    res = bass_utils.run_bass_kernel_spmd(
        nc, in_maps, core_ids=list(range(C)))

    out_user = np.empty((n_user, P), np.float32)
    out_item = np.empty((n_item, P), np.float32)
    for c in range(C):
        ou = np.asarray(res.results[c]["out_user"], np.float32)
        oi = np.asarray(res.results[c]["out_item"], np.float32)
        out_user[c * ru + ord_u[c]] = ou[:ru]
        out_item[c * ri + ord_i[c]] = oi[:ri]
    return out_user, out_item
